# revision 33
# baseline (speedup 1.0000x reference)
"""Trainium2 Bass kernel: batched 1-D linear interpolation on a uniform grid.

out[b, j] = y[b, i_j] + w_j * (y[b, i_j + 1] - y[b, i_j])

i_j / w_j depend only on x_new, so the host folds them into a sparse
selection-matrix S [NUM_POINTS, M] with exactly two nonzeros per column
((1-w_j) at row i_j, w_j at row i_j+1).  The dense out = y @ S matmul of the
previous version burned ~437 us of PE per core (16 contraction chunks per
output column when only one holds the two nonzeros), so this version exploits
the sparsity structure:

- Output columns are processed in i_j-sorted order (a pure layout permutation,
  undone on the host during unshard, like the host-side y transpose).
- The grid is cut into 17 chunks of STRIDE 127 (chunk c = floor(i/127)); a
  column with i in chunk c has i+1 <= 127c+127 also inside the chunk's 128
  loaded rows, so every output column is produced by exactly ONE K=128 matmul.
  Sorted columns make each chunk's output range contiguous, so per 128-row
  batch tile the whole M=4096 output is ~24 single-shot matmuls (17 chunks
  split at PSUM bank boundaries) -- 4096 moving columns instead of the dense
  65536.
- Everything 2-byte: yT and S ship as fp16 and the output is stored as fp16
  (PSUM accumulates in fp32; the PSUM->SBUF copy downcasts) and upcast to
  fp32 on the host.  Per-core HBM traffic: 8.9 MiB yT + 1 MiB S + 16 MiB out.

Sharding: pure data parallel over the batch axis across 8 NeuronCores
(y_points rows 16384 -> 8 x 2048); x_new-derived constants are replicated.
"""

import numpy as np

BATCH = 16384
NUM_POINTS = 2048
M = 4096
N_CORES = 8
ROWS_PER_CORE = BATCH // N_CORES  # 2048
P = 128
N_BTILES = ROWS_PER_CORE // P  # 16 batch tiles per core
CH = 127  # grid chunk stride; 128 loaded rows per chunk -> i and i+1 colocate
N_CHUNKS = 17  # ceil((NUM_POINTS-1)/CH): chunks cover i in [0, 2046]
N_BANKS = 8  # PSUM banks; 8 x 512 fp32 = M
BANK = M // N_BANKS  # 512

_NC_CACHE = {}


def _build_nc(widths):
    """Program parameterized by the per-chunk sorted-column counts (sum = M)."""
    import concourse.bacc as bacc
    import concourse.mybir as mybir
    from concourse.tile import TileContext

    f32 = mybir.dt.float32
    f16 = mybir.dt.float16

    offs = [0]
    for wd in widths:
        offs.append(offs[-1] + int(wd))
    assert offs[-1] == M

    # Matmul pieces: chunk ranges split at PSUM bank boundaries so each piece
    # is a single start&stop matmul into one bank.  Pieces are grouped into
    # four quarter-units of 2 banks (1024 cols) each: every unit has its own
    # 2-bank PSUM tile drained by a SINGLE 1024-wide copy (one reader per
    # tile -- two readers of one tile serialize), engines alternate per unit,
    # and the WAR distance back to the same PSUM tile is 4 units, which hides
    # the ~0.7 us semaphore-increment latency that a whole-btile group
    # structure paid once per btile.
    QW = M // 4  # 1024 columns per quarter-unit
    by_unit = [[] for _ in range(4)]
    for c in range(N_CHUNKS):
        lo, hi = offs[c], offs[c + 1]
        while lo < hi:
            bend = min(hi, (lo // BANK + 1) * BANK)
            by_unit[lo // QW].append((c, lo, bend))
            lo = bend

    nc = bacc.Bacc()
    # yT[p, b, c, q] = y[128*b + q, 127*c + p] as fp16 (0 where 127c+p >= 2048)
    yT = nc.dram_tensor("yT", [P, N_BTILES * N_CHUNKS * P], f16, kind="ExternalInput")
    # s[r, t] = packed selection matrix: sorted column t (chunk c_t) has
    # (1-w) at r = i - 127*c_t and w at r+1.
    s = nc.dram_tensor("s", [P, M], f16, kind="ExternalInput")
    # out[q, t] = fp16 result for sorted column t (host unpermutes + upcasts)
    out = nc.dram_tensor("out", [ROWS_PER_CORE, M], f16, kind="ExternalOutput")

    with TileContext(nc) as tc:
        with (
            tc.tile_pool(name="const", bufs=1) as cp,
            tc.tile_pool(name="psum", bufs=1, space="PSUM") as pp,
            tc.tile_pool(name="outp", bufs=4) as op,
        ):
            yT_t = cp.tile([P, N_BTILES, N_CHUNKS, P], f16, tag="yT")
            s_t = cp.tile([P, M], f16, tag="s")
            # s quarters interleaved with the first yT blocks at the head of
            # the fast sync ring (the scalar/Activation ring measures ~70
            # GB/s -- far too slow for bulk): unit u of btile 0 gates on
            # s quarter u + yT block 0 only, so the PE starts ~10.4 us in
            # with no per-unit stalls.  (s off gpsimd also keeps the store
            # ring clean.)
            def load_yt(b, c0=0, c1=N_CHUNKS):
                nc.sync.dma_start(
                    out=yT_t[:, b, c0:c1],
                    in_=yT[
                        :, b * N_CHUNKS * P + c0 * P : b * N_CHUNKS * P + c1 * P
                    ].rearrange("p (c q) -> p c q", c=c1 - c0),
                )

            def load_s(q):
                nc.sync.dma_start(
                    out=s_t[:, q * QW : (q + 1) * QW], in_=s[:, q * QW : (q + 1) * QW]
                )

            # btile 0's unit 0 only reads the low grid chunks (sorted columns
            # [0,1024) map to low i), so a ~160 KiB head piece of yT block 0
            # right after s quarter 0 lets the PE start ~2 us earlier than
            # waiting for the whole 544 KiB block.
            head_chunks = max(c for c, _, _ in by_unit[0]) + 1
            load_s(0)
            load_yt(0, 0, head_chunks)
            load_s(1)
            load_yt(0, head_chunks, N_CHUNKS)
            load_s(2)
            load_yt(1)
            load_s(3)
            for b in range(2, N_BTILES):
                load_yt(b)
            # (No PE pre-warm: K=1 dummy matmuls measurably do NOT trip the
            # HAM activity monitor -- real matmuls still started at 1.2 GHz
            # -- and a dummy chain long enough to bridge to data-ready just
            # delays the first real matmul behind it on the PE FIFO.)

            H = M // 2  # 2048 columns per half-btile store
            for b in range(N_BTILES):
                o_t = None
                for u in range(4):
                    if u % 2 == 0:
                        o_t = op.tile([P, H], f16, tag="o", name="o_t")
                    ps = pp.tile([P, QW], f32, tag=f"ps{u}", name=f"ps{u}")
                    for c, lo, hi in by_unit[u]:
                        nc.tensor.matmul(
                            ps[:, lo - u * QW : hi - u * QW],
                            yT_t[:, b, c, :],
                            s_t[:, lo:hi],
                            start=True,
                            stop=True,
                        )
                    # One 1024-wide drain per unit, engines alternating; fp32
                    # PSUM -> fp16 SBUF downcast happens here for free.
                    dst = o_t[:, (u % 2) * QW : (u % 2 + 1) * QW]
                    if u % 2 == 0:
                        nc.vector.tensor_copy(out=dst, in_=ps[:])
                    else:
                        nc.scalar.copy(out=dst, in_=ps[:])
                    # gpsimd ring: keeps the 16 MiB store stream off the
                    # input-laden sync ring.  The last btile stores per unit
                    # so the final copy->store->drain tail is a quarter-store
                    # deep instead of a half.
                    if b == N_BTILES - 1:
                        nc.gpsimd.dma_start(
                            out=out[b * P : (b + 1) * P, u * QW : (u + 1) * QW],
                            in_=o_t[:, (u % 2) * QW : (u % 2 + 1) * QW],
                        )
                    elif u % 2 == 1:
                        h = u // 2
                        nc.gpsimd.dma_start(
                            out=out[b * P : (b + 1) * P, h * H : (h + 1) * H],
                            in_=o_t[:],
                        )

    nc.compile()
    return nc


def _get_nc(widths):
    key = tuple(int(w) for w in widths)
    if key not in _NC_CACHE:
        _NC_CACHE[key] = _build_nc(key)
    return _NC_CACHE[key]


def _host_precompute(x_new):
    """Replicate the reference's searchsorted/weight math with the same jax
    ops on the same backend, so boundary decisions and weight rounding match
    the reference bit-for-bit (the device searchsorted/divide are not IEEE-
    exact, so numpy does NOT reproduce them)."""
    import jax.numpy as jnp

    x_new_j = jnp.asarray(np.asarray(x_new, dtype=np.float32))
    x_points = jnp.linspace(0.0, 1.0, NUM_POINTS, dtype=x_new_j.dtype)
    idxs = jnp.searchsorted(x_points, x_new_j, side="right") - 1
    idxs = jnp.clip(idxs, 0, NUM_POINTS - 2)
    x1 = x_points[idxs]
    x2 = x_points[idxs + 1]
    w = (x_new_j - x1) / (x2 - x1)
    return np.asarray(idxs).astype(np.int64), np.asarray(w, dtype=np.float32)


def _plan(x_new):
    idxs, w = _host_precompute(np.asarray(x_new))
    order = np.argsort(idxs, kind="stable")  # sorted-column -> original column
    si = idxs[order]
    chunk_of = si // CH  # non-decreasing, in [0, N_CHUNKS)
    widths = np.bincount(chunk_of, minlength=N_CHUNKS)
    # Packed selection matrix in sorted order.
    r = (si - chunk_of * CH).astype(np.int64)  # row within chunk, [0, 126]
    t = np.arange(M)
    S = np.zeros((P, M), dtype=np.float32)
    sw = w[order]
    S[r, t] = 1.0 - sw
    S[r + 1, t] = sw
    return order, widths, S.astype(np.float16)


def _make_in_maps(y_points, s_pack):
    y_full = np.asarray(y_points, dtype=np.float32)
    # yT_all[core, p, b, c, q] = y_full[2048*core + 128*b + q, 127*c + p]
    yT_all = np.zeros((N_CORES, P, N_BTILES, N_CHUNKS, P), dtype=np.float16)
    for c in range(N_CHUNKS):
        g0 = CH * c
        g1 = min(g0 + P, NUM_POINTS)
        n = g1 - g0
        blk = y_full[:, g0:g1].T.astype(np.float16)  # [n, BATCH]
        yT_all[:, :n, :, c, :] = blk.reshape(n, N_CORES, N_BTILES, P).transpose(
            1, 0, 2, 3
        )
    return [
        {"yT": yT_all[core].reshape(P, N_BTILES * N_CHUNKS * P), "s": s_pack}
        for core in range(N_CORES)
    ]


def run(y_points, x_new, trace=False, **spmd_kwargs):
    """Run the Bass kernel; returns (output, BassKernelResults)."""
    from concourse.bass_utils import run_bass_kernel_spmd

    order, widths, s_pack = _plan(x_new)
    nc = _get_nc(widths)
    in_maps = _make_in_maps(y_points, s_pack)
    res = run_bass_kernel_spmd(
        nc, in_maps, list(range(N_CORES)), trace=trace, **spmd_kwargs
    )
    out_sorted = np.concatenate([r["out"] for r in res.results], axis=0)
    out = np.empty((BATCH, M), dtype=np.float32)
    out[:, order] = out_sorted  # unpermute + fp16 -> fp32 upcast
    return out, res


def kernel(y_points, x_new):
    out, _ = run(y_points, x_new)
    return out
